# revision 26
# baseline (speedup 1.0000x reference)
"""MultiHeadAttention (B=2, S=2048, D=2048, H=16, RoPE) on 8 NeuronCores.

Sharding: tensor-parallel over heads. Core c owns heads 2c, 2c+1 (256 channels).
Each core: QKV projections for its channels, RoPE, full attention for its 2
heads, and a partial output projection y_c = ctx_c @ Wo[:, ch_c].T. Host sums
the 8 partials (fp16 partials, fp32 sum).

All-fp16 compute with fp32 PSUM accumulation:
  - Host pre-rearranges x and weights into partition-major layouts so every
    DMA moves partition-contiguous 8-16KB runs (128 descriptors/DMA; large
    strided loads would pay ~3.5ns/descriptor in HWDGE issue otherwise).
  - Projections: per 512-token chunk, v-sweep (stationary x subtiles) then
    qk-sweep (stationary weight tiles). PSUM: q 2 + k 2 banks single-buffered
    + v 4 banks (each of the 4 token-subtile accumulation groups owns a FULL
    bank: start=True clears has_written for the whole bank, so groups must
    not share one). x streams on the gpsimd SWDGE ring in parallel with
    weight loads on the sync ring. q/k staged to fp16 SBUF via ScalarE
    copies, RoPE on VectorE in fp16 2x mode (sin table laid out [s; -s] so
    each half-product reads both inputs at one base partition), v evicted on
    ScalarE.
  - Attention (transposed scores): software pipeline at kt-pair granularity:
    scores+exp for chunk i interleave with PV/denominator matmuls for chunk
    i-1, so the PE always has exp-independent work. exp runs on 1024-element
    2-bank ACTIVATEs. The denominator uses a full [128,128] ones stationary:
    its PSUM accumulator IS the denominator broadcast to all partitions
    (a [128,1] ones stationary would be a partial-column weight load that
    breaks background weight-load pipelining and costs ~93ns on neighboring
    matmuls), then reciprocal_approx_fast + one tensor_tensor mul normalize
    into fp16 ctxT.
  - Output projection trails the pipeline by one more chunk (the normalize
    needs ~1.5us of DVE after the last PV matmul) and is emitted first in
    each iteration; y evictions split across VectorE and ScalarE; y written
    as fp16, one DMA per 128-token row block.
"""
import sys

sys.path.insert(0, "/opt/trn_rl_repo")

import numpy as np

B, S, D, H = 2, 2048, 2048, 16
HD = D // H          # 128
NCORES = 8
HPC = H // NCORES    # heads per core
CPC = HPC * HD       # channels per core = 256
TOK = B * S          # 4096
P = 128
KT = D // P          # 16 contraction tiles
NCH = 512            # token chunk for projections / attention qtok chunk
ROPE_BASE = 10000.0

_cache = {}


def _build_nc():
    import concourse.bass as bass  # noqa: F401
    import concourse.mybir as mybir
    import concourse.tile as tile
    from concourse import bacc

    F32 = mybir.dt.float32
    F16 = mybir.dt.float16
    AF = mybir.ActivationFunctionType
    MUL = mybir.AluOpType.mult
    ADD = mybir.AluOpType.add

    nc = bacc.Bacc(None, target_bir_lowering=False)

    NQC = TOK // NCH            # 8 projection chunks
    SQC = S // NCH              # 4 attention q-chunks per sequence
    SKT = S // P                # 16 key tiles per sequence
    VST = NCH // P              # 4 v subtiles per chunk
    HF = HD // 2                # 64
    SCALE = 1.0 / float(np.sqrt(HD))

    # host-rearranged inputs: partition-major, contiguous per partition
    xT_d = nc.dram_tensor("xR", [P, NQC, KT, NCH], F16, kind="ExternalInput")
    wq_d = nc.dram_tensor("wqR", [P, KT, CPC], F16, kind="ExternalInput")
    wk_d = nc.dram_tensor("wkR", [P, KT, CPC], F16, kind="ExternalInput")
    wv_d = nc.dram_tensor("wvR", [P, KT, CPC], F16, kind="ExternalInput")
    wo_d = nc.dram_tensor("woR", [P, HPC, D], F16, kind="ExternalInput")
    cos_d = nc.dram_tensor("cos2", [P, S], F16, kind="ExternalInput")
    sin_d = nc.dram_tensor("sin2", [P, S], F16, kind="ExternalInput")
    ones_d = nc.dram_tensor("ones128", [P, P], F16, kind="ExternalInput")
    y_d = nc.dram_tensor("y", [TOK, D], F16, kind="ExternalOutput")

    with tile.TileContext(nc) as tc, \
         nc.allow_low_precision(reason="fp16 compute, fp32 accumulate"):
        with tc.tile_pool(name="persist", bufs=1) as pp_:
            # long-lived tensors
            qT = [pp_.tile([P, TOK], F16, name=f"qT{m}") for m in range(HPC)]
            kTt = [pp_.tile([P, TOK], F16, name=f"kT{m}") for m in range(HPC)]
            vS = pp_.tile([P, TOK // P, CPC], F16, name="vS")

            wq = pp_.tile([P, KT, CPC], F16, name="wq")
            wk = pp_.tile([P, KT, CPC], F16, name="wk")
            wv = pp_.tile([P, KT, CPC], F16, name="wv")
            cos2 = pp_.tile([P, S], F16, name="cos2")
            sin2 = pp_.tile([P, S], F16, name="sin2")
            wo = pp_.tile([P, HPC, D], F16, name="wo")
            ones128 = pp_.tile([P, P], F16, name="ones128")
            nc.sync.dma_start(wv[:], wv_d[:])

            # warm the exp table set so ACT_TABLE_LOAD is off the critical path
            warm_in = pp_.tile([1, 1], F32, name="warm_in")
            warm = pp_.tile([1, 1], F32, name="warm")
            nc.vector.memset(warm_in[:], 0.0)
            nc.scalar.activation(warm[:], warm_in[:], AF.Exp)

            # ~45 dummy matmuls on a zeroed tile while the first DMAs land:
            # keeps the PE HAM busy past its 3.4us window so real matmuls
            # start at 2.4GHz instead of 1.2
            wt16 = pp_.tile([P, P], F16, name="wt16")
            nc.vector.memset(wt16[:], 0.0)
            with tc.tile_pool(name="wrm", bufs=1, space="PSUM") as wrmp:
                wrm_ps = wrmp.tile([P, P], F32, name="wrm_ps")
                for _ in range(135):
                    nc.tensor.matmul(wrm_ps[:], wt16[:], wt16[:],
                                     start=True, stop=True)

            # ---------------- Phase 1: projections + RoPE ----------------
            with tc.tile_pool(name="xp", bufs=6) as xp, \
                 tc.tile_pool(name="stg", bufs=4) as stg, \
                 tc.tile_pool(name="rp", bufs=4) as rp, \
                 tc.tile_pool(name="qkp", bufs=1, space="PSUM") as qkp, \
                 tc.tile_pool(name="vp", bufs=1, space="PSUM") as vp:
                for ch in range(NQC):
                    t0 = ch * NCH
                    s0 = (ch % SQC) * NCH  # position within sequence
                    # x for this chunk: [128, 16, 512] as two half DMAs on
                    # the SWDGE ring (parallel to weight loads on sync)
                    xt = [xp.tile([P, KT // 2, NCH], F16, name="xt") for _ in range(2)]
                    if ch == 0:
                        nc.sync.dma_start(xt[0][:], xT_d[:, 0, 0:8, :])
                        nc.sync.dma_start(xt[1][:], xT_d[:, 0, 8:16, :])
                        nc.sync.dma_start(wq[:], wq_d[:])
                        nc.sync.dma_start(wk[:], wk_d[:])
                        nc.sync.dma_start(cos2[:], cos_d[:])
                        nc.sync.dma_start(sin2[:], sin_d[:])
                        nc.sync.dma_start(wo[:], wo_d[:])
                        nc.sync.dma_start(ones128[:], ones_d[:])
                    else:
                        # alternate rings: each carries ~50 GB/s, far from
                        # the ~90 GB/s per-ring cliff
                        eng = nc.gpsimd if ch % 2 else nc.sync
                        for h in range(2):
                            eng.dma_start(
                                xt[h][:], xT_d[:, ch, h * 8:(h + 1) * 8, :]
                            )

                    # ---- v-sweep: stationary x subtiles, moving wv ----
                    v_ps = vp.tile([P, VST, 512], F32, name="v_ps")
                    for kt in range(KT):
                        xtile = xt[kt // 8][:, kt % 8, :]
                        st_, sp_ = (kt == 0), (kt == KT - 1)
                        for st in range(VST):
                            nc.tensor.matmul(
                                v_ps[:, st, 0:CPC],
                                xtile[:, st * P:(st + 1) * P],
                                wv[:, kt, :],
                                start=st_, stop=sp_,
                            )
                    # evict v on ScalarE (fp32 psum -> fp16 sbuf)
                    nc.scalar.activation(
                        vS[:, ch * VST:(ch + 1) * VST, :], v_ps[:, :, 0:CPC],
                        AF.Copy,
                    )

                    # ---- qk-sweep: stationary weights, moving x ----
                    q_ps = qkp.tile([P, HPC, NCH], F32, name="q_ps")
                    k_ps = qkp.tile([P, HPC, NCH], F32, name="k_ps")
                    for kt in range(KT):
                        xtile = xt[kt // 8][:, kt % 8, :]
                        st_, sp_ = (kt == 0), (kt == KT - 1)
                        for m in range(HPC):
                            nc.tensor.matmul(
                                q_ps[:, m, :], wq[:, kt, m * P:(m + 1) * P],
                                xtile[:], start=st_, stop=sp_,
                            )
                            nc.tensor.matmul(
                                k_ps[:, m, :], wk[:, kt, m * P:(m + 1) * P],
                                xtile[:], start=st_, stop=sp_,
                            )
                    # stage q/k to SBUF fp16 on ScalarE (frees PSUM fast)
                    qsb = stg.tile([P, HPC, NCH], F16, name="qsb")
                    ksb = stg.tile([P, HPC, NCH], F16, name="ksb")
                    nc.scalar.activation(qsb[:], q_ps[:], AF.Copy)
                    nc.scalar.activation(ksb[:], k_ps[:], AF.Copy)

                    # RoPE on VectorE in fp16: out = src*cos2 + swap(src)*sin2
                    # sin2 is laid out [s; -s] so each half-product reads its
                    # inputs at a shared base partition (DVE requirement) and
                    # only the OUTPUT lands in the opposite half:
                    #   rot[0:64]   = src[64:128]*sin2[64:128]  (= -s half)
                    #   rot[64:128] = src[0:64]  *sin2[0:64]    (= +s half)
                    for m in range(HPC):
                        for src, dst in ((qsb, qT[m]), (ksb, kTt[m])):
                            sm = src[:, m, :]
                            rot = rp.tile([P, NCH], F16, name="rot")
                            nc.vector.tensor_tensor(
                                rot[0:HF, :], sm[HF:P, :],
                                sin2[HF:P, s0:s0 + NCH], MUL,
                            )
                            nc.vector.tensor_tensor(
                                rot[HF:P, :], sm[0:HF, :],
                                sin2[0:HF, s0:s0 + NCH], MUL,
                            )
                            tmp = rp.tile([P, NCH], F16, name="tmp")
                            nc.vector.tensor_tensor(
                                tmp[:], sm[:], cos2[:, s0:s0 + NCH], MUL
                            )
                            nc.vector.tensor_tensor(
                                dst[:, t0:t0 + NCH], tmp[:], rot[:], ADD
                            )

            # ---------------- Phase 2+3: attention + output projection ----
            with tc.tile_pool(name="ep", bufs=3) as ep, \
                 tc.tile_pool(name="esp", bufs=3) as esp, \
                 tc.tile_pool(name="dp", bufs=4) as dp, \
                 tc.tile_pool(name="yp", bufs=6) as yp, \
                 tc.tile_pool(name="ctxp", bufs=1) as ctxp, \
                 tc.tile_pool(name="sp2", bufs=2, space="PSUM") as spsum, \
                 tc.tile_pool(name="cp", bufs=1, space="PSUM") as cpsum, \
                 tc.tile_pool(name="ap", bufs=2, space="PSUM") as apsum:
                ctxT = [
                    ctxp.tile([P, S], F16, name=f"ctxT{b}_{m}")
                    for b in range(B)
                    for m in range(HPC)
                ]

                def oproj_part(b, qc):
                    # project the 4 row-blocks of this qtok chunk (both
                    # heads' ctxT slices are final by now)
                    for tt in range(qc * NCH // P, (qc + 1) * NCH // P):
                        row0 = b * S + tt * P
                        y_sb = yp.tile([P, D], F16, name="y_sb")
                        for nck in range(D // NCH):
                            y_ps = apsum.tile([P, NCH], F32, name="y_ps")
                            for m in range(HPC):
                                nc.tensor.matmul(
                                    y_ps[:],
                                    ctxT[b * HPC + m][:, tt * P:(tt + 1) * P],
                                    wo[:, m, nck * NCH:(nck + 1) * NCH],
                                    start=(m == 0), stop=(m == HPC - 1),
                                )
                            if nck < 2 or (nck == 2 and tt % 2 == 0):
                                nc.vector.tensor_copy(
                                    y_sb[:, nck * NCH:(nck + 1) * NCH],
                                    y_ps[:],
                                )
                            else:
                                nc.scalar.activation(
                                    y_sb[:, nck * NCH:(nck + 1) * NCH],
                                    y_ps[:], AF.Copy,
                                )
                        nc.sync.dma_start(y_d[row0:row0 + P, :], y_sb[:])

                def fused_iter(cur, prev, ex_prev, es_prev, oproj_q):
                    """Interleave scores+exp for `cur` with PV/den for `prev`
                    at kt-pair granularity; emit any queued output-projection
                    chunk first (its normalize finished an iteration ago)."""
                    if oproj_q:
                        oproj_part(*oproj_q.pop(0))
                    ex_new = None
                    es_new = None
                    if cur is not None:
                        bC, mC, qcC = cur
                        qt0 = bC * S + qcC * NCH
                        ex_new = ep.tile([P, SKT, NCH], F16, name="ex")
                        es_new = esp.tile([P, NCH], F16, name="es")
                    if prev is not None:
                        bP, mP, qcP = prev
                        ctx_ps = cpsum.tile([P, NCH], F32, name="ctx_ps")
                        # full-bank accumulator; every row = the denominator
                        den_ps = cpsum.tile([P, NCH], F32, name="den_ps")
                    for kp in range(SKT // 2):
                        if cur is not None:
                            scr = spsum.tile([P, 2, NCH], F32, name="scr")
                            for j in range(2):
                                kt = kp * 2 + j
                                nc.tensor.matmul(
                                    scr[:, j, :],
                                    kTt[mC][:, bC * S + kt * P:
                                             bC * S + (kt + 1) * P],
                                    qT[mC][:, qt0:qt0 + NCH],
                                    start=True, stop=True,
                                )
                            nc.scalar.activation(
                                ex_new[:, kp * 2:kp * 2 + 2, :], scr[:],
                                AF.Exp, scale=SCALE,
                            )
                            # running kt-sum of ex (kt 0..11) on VectorE:
                            # 16 denominator matmuls become 5 (1 exsum + 4)
                            if kp == 0:
                                nc.vector.tensor_tensor(
                                    es_new[:], ex_new[:, 0, :],
                                    ex_new[:, 1, :], ADD,
                                )
                            elif kp < 6:
                                for j in range(2):
                                    nc.vector.tensor_tensor(
                                        es_new[:], es_new[:],
                                        ex_new[:, kp * 2 + j, :], ADD,
                                    )
                        if prev is not None:
                            for j in range(2):
                                kt = kp * 2 + j
                                gkt = bP * SKT + kt
                                st_, sp_ = (kt == 0), (kt == SKT - 1)
                                nc.tensor.matmul(
                                    ctx_ps[:],
                                    vS[:, gkt, mP * P:(mP + 1) * P],
                                    ex_prev[:, kt, :],
                                    start=st_, stop=sp_,
                                )

                    if prev is not None:
                        nc.tensor.matmul(
                            den_ps[:], ones128[:], es_prev[:],
                            start=True, stop=False,
                        )
                        for kt in range(12, SKT):
                            nc.tensor.matmul(
                                den_ps[:], ones128[:], ex_prev[:, kt, :],
                                start=False, stop=(kt == SKT - 1),
                            )
                        rec = dp.tile([P, NCH], F32, name="rec")
                        nc.vector.reciprocal_approx_fast(out=rec[:], in_=den_ps[:])
                        nc.vector.tensor_tensor(
                            ctxT[bP * HPC + mP][:, qcP * NCH:(qcP + 1) * NCH],
                            ctx_ps[:], rec[:], MUL,
                        )
                        if mP == HPC - 1:
                            oproj_q.append((bP, qcP))
                    return ex_new, es_new

                work = [(b, m, qc) for b in range(B) for m in range(HPC)
                        for qc in range(SQC)]
                ex_prev = es_prev = None
                oproj_q = []
                for i in range(len(work) + 1):
                    cur = work[i] if i < len(work) else None
                    prev = work[i - 1] if i > 0 else None
                    ex_prev, es_prev = fused_iter(
                        cur, prev, ex_prev, es_prev, oproj_q)
                while oproj_q:
                    oproj_part(*oproj_q.pop(0))
    nc.finalize()
    return nc


def _rope_tables():
    inv_freq = (1.0 / (ROPE_BASE ** (np.arange(0, HD, 2, dtype=np.float32) / HD))).astype(np.float32)
    t = np.arange(S, dtype=np.float32)
    freqs = np.outer(t, inv_freq).astype(np.float32)  # [S, HD/2]
    c = np.cos(freqs).astype(np.float32).T            # [64, S]
    s = np.sin(freqs).astype(np.float32).T
    cos2 = np.concatenate([c, c], axis=0)             # [128, S]
    sin2 = np.concatenate([s, -s], axis=0)            # [128, S]: [s; -s]
    return np.ascontiguousarray(cos2), np.ascontiguousarray(sin2)


def kernel(x, Wq, Wk, Wv, Wo):
    from concourse.bass_utils import run_bass_kernel_spmd

    F16 = np.float16
    NQC = TOK // NCH

    x = np.asarray(x, dtype=np.float32)
    Wq = np.asarray(Wq, dtype=np.float32)
    Wk = np.asarray(Wk, dtype=np.float32)
    Wv = np.asarray(Wv, dtype=np.float32)
    Wo = np.asarray(Wo, dtype=np.float32)

    # x rearranged to [p, chunk, kt, token]: partition-contiguous DMA runs
    xR = np.ascontiguousarray(
        x.reshape(NQC, NCH, KT, P).transpose(3, 0, 2, 1).astype(F16)
    )
    cos2, sin2 = _rope_tables()
    cos2 = np.ascontiguousarray(cos2.astype(F16))
    sin2 = np.ascontiguousarray(sin2.astype(F16))
    ones128 = np.ones((P, P), dtype=F16)

    def wslices(W, c):  # [D, CPC] -> [p, kt, cpc]
        ch0, ch1 = c * CPC, (c + 1) * CPC
        wT = W[ch0:ch1, :].T  # [D, CPC]
        return np.ascontiguousarray(
            wT.reshape(KT, P, CPC).transpose(1, 0, 2).astype(F16)
        )

    in_maps = []
    for c in range(NCORES):
        ch0, ch1 = c * CPC, (c + 1) * CPC
        woT = Wo[:, ch0:ch1].T  # [CPC, D]
        in_maps.append({
            "xR": xR,
            "wqR": wslices(Wq, c),
            "wkR": wslices(Wk, c),
            "wvR": wslices(Wv, c),
            "woR": np.ascontiguousarray(
                woT.reshape(HPC, P, D).transpose(1, 0, 2).astype(F16)
            ),
            "cos2": cos2,
            "sin2": sin2,
            "ones128": ones128,
        })

    if "nc" not in _cache:
        _cache["nc"] = _build_nc()
    res = run_bass_kernel_spmd(_cache["nc"], in_maps, core_ids=list(range(NCORES)))
    _cache["last_results"] = res

    y = np.zeros((TOK, D), dtype=np.float32)
    for rm in res.results:
        y += rm["y"].astype(np.float32)
    return y.reshape(B, S, D)
